# revision 17
# baseline (speedup 1.0000x reference)
"""Trainium2 Bass kernel: CorDBN (ZCA channel whitening) over X[128, 64, 56, 56].

Math: with x = X viewed as [C=64, m=B*H*W], the op is
    out = wm @ ((x - mean) / std)
where std is the per-channel (ddof=1) std + 1e-5, sigma = eps*I + corr/m and
wm = sigma^{-1/2}.  This is a per-column affine map out = A @ x + b with
    A = wm @ diag(1/std),    b = -wm @ (mean/std).

Plan (8 cores, data-parallel over batch, 16 batches per core):
  phase 1: DMA two-batch tiles [128, 3136] into SBUF (kept resident),
           PE-transpose 128-column slices (fp32r: 1.5 cyc/row), accumulate
           the Gram via ONE matmul per chunk: lhsT=[w,128] (A|B halves),
           rhs=[w,129] (A|B|ones) -> PSUM [128,129]; diagonal 64x64 blocks
           are the per-half Grams, col 128 the per-channel sums.
  stats:   fold the two halves, AllReduce the [64, 65] payload across
           cores; compute sigma and run Newton-Schulz on-device for
           wm = sigma^{-1/2}; build block-diag lhsT [A^T|A^T] and bias.
  phase 2: one fp32r matmul per [128, 512] chunk against the resident
           fp32 tiles (fp32r streams 1 row/cycle at N>=256 vs fp32's 4),
           bias added during the PSUM->SBUF copy (scalar/vector/gpsimd
           rotation), DMA out.
"""
import numpy as np

import concourse.bass as bass
import concourse.tile as tile
from concourse import mybir
from concourse.bass_utils import run_bass_kernel_spmd
from concourse.vector_clock import ScopedClock

# ---------------- problem constants (hardcoded: must be self-contained) ----
B, C, H, W = 128, 64, 56, 56
HW = H * W                      # 3136
N_CORES = 8
B_LOC = B // N_CORES            # 16 batches per core
PAIRS = B_LOC // 2              # 8 two-batch tiles per core
M_TOT = B * HW                  # 401408
EPS = 1e-3
EPS_BN = 1e-5
NS_ITERS = 2
F32 = mybir.dt.float32
F32R = mybir.dt.float32r
BF16 = mybir.dt.bfloat16

TCH = 128                       # transpose chunk width (phase 1)
N_FULL = HW // TCH              # 24
REM = HW - N_FULL * TCH         # 64
OCH = 512                       # phase-2 output chunk width
P2_CHUNKS = [(i * OCH, OCH) for i in range(HW // OCH)] + [
    (HW - HW % OCH, HW % OCH)
]  # 6 x 512 + 1 x 64


# ---------------- old-walrus workaround: 1 sync wait per instruction -------
# This walrus build rejects instructions carrying more than one sem wait
# ("Too many sync wait commands").  Split: excess waits move onto fresh
# same-engine nops placed immediately before the instruction.
_MAXW = 1

_orig_commit_and_lower = tile.TileContext._commit_and_lower


def _commit_and_lower_split(self, inst, bb, old_bb_map, bb_to_exit_bb):
    si = inst.sync_info
    if si is not None and len(si.on_wait) > _MAXW:
        waits = list(si.on_wait)
        excess = waits[:-_MAXW]
        del si.on_wait[:len(waits) - _MAXW]
        eng = self.nc.engines[inst.engine]
        for i in range(0, len(excess), _MAXW):
            nop = eng.nop(nofuse=True, hint="split_wait")
            nop.ins.sync_info = mybir.SyncInfo(
                on_wait=list(excess[i:i + _MAXW]), on_update=[]
            )
    return _orig_commit_and_lower(self, inst, bb, old_bb_map, bb_to_exit_bb)


tile.TileContext._commit_and_lower = _commit_and_lower_split


def _drain_and_barrier_split(self, tick_clock, wait_clock):
    MAXW = _MAXW
    probe = self.nc.sync.drain()
    wait_clock.add_sem_waits(probe.ins, ScopedClock({None: tick_clock.global_clock}))
    if probe.ins.sync_info is None:
        probe.ins.sync_info = mybir.SyncInfo(on_wait=[], on_update=[])
    n = len(probe.ins.sync_info.on_wait)
    del probe.ins.sync_info.on_wait[MAXW:]
    for start in range(MAXW, n, MAXW):
        extra = self.nc.sync.drain()
        wait_clock.add_sem_waits(
            extra.ins, ScopedClock({None: tick_clock.global_clock})
        )
        si = extra.ins.sync_info
        del si.on_wait[start + MAXW:]
        del si.on_wait[:start]
    self.nc.all_engine_barrier()
    popped = self.nc._tile_sem_poison_stack.pop()
    assert popped is self._sem_poison
    self.nc.clear_and_free_semaphores(list(self.sems.allocated().values()))
    self.nc.all_engine_barrier()


tile.TileContext._drain_and_barrier = _drain_and_barrier_split


def build_bass(repeat: int = 1, use_collective: bool = True):
    nc = bass.Bass("TRN2", target_bir_lowering=False, debug=False,
                   num_devices=N_CORES)
    # X and IDENT are declared float32r (bit-identical storage to fp32) so
    # the fp32r tensor-engine path (1 cycle/row at N>=256 vs fp32's 4) sees
    # legally-"rounded" producers end to end.
    X = nc.dram_tensor("X", [B_LOC, C, HW], F32R, kind="ExternalInput").ap()
    OUT = nc.dram_tensor("OUT", [B_LOC, C, HW], F32, kind="ExternalOutput").ap()
    IDENT = nc.dram_tensor("IDENT", [128, 128], F32R,
                           kind="ExternalInput").ap()
    EYE3 = nc.dram_tensor("EYE3", [C, C], F32, kind="ExternalInput").ap()
    EPSEYE = nc.dram_tensor("EPSEYE", [C, C], F32, kind="ExternalInput").ap()

    cc_in = nc.dram_tensor("cc_in", [C, C + 1], F32)
    cc_out = nc.dram_tensor("cc_out", [C, C + 1], F32, addr_space="Shared")

    with tile.TileContext(nc) as tc:
        with (
            tc.tile_pool(name="const", bufs=1) as cpool,
            tc.tile_pool(name="xres", bufs=1) as xpool,
            tc.tile_pool(name="tsb", bufs=1) as tsbpool,
            tc.tile_pool(name="small", bufs=1) as spool,
        ):
            ident_sb = cpool.tile([128, 128], F32R, tag="ident")
            nc.gpsimd.dma_start(out=ident_sb[:], in_=IDENT)
            eye3_sb = cpool.tile([C, C], F32, tag="eye3")
            nc.gpsimd.dma_start(out=eye3_sb[:], in_=EYE3)
            epseye_sb = cpool.tile([C, C], F32, tag="epseye")
            nc.gpsimd.dma_start(out=epseye_sb[:], in_=EPSEYE)
            eyec = ident_sb[0:C, 0:C].bitcast(F32)

            # transposed-chunk staging tiles (bf16), manual ring of 4.
            # layout per tile: 4 blocks of 130 cols; block j holds the
            # chunk's 128 transposed data cols at [130j, 130j+128), a ones
            # column at 130j+128, and a pad col at 130j+129.
            tsb_tiles = []
            for i in range(4):
                t = tsbpool.tile([128, 520], BF16, tag=f"tsb{i}", name=f"tsb{i}")
                for j in range(4):
                    nc.vector.memset(t[:, 130 * j + 128:130 * j + 129], 1.0)
                tsb_tiles.append(t)

            for _rep in range(repeat):
                run_one_pass(nc, tc, cpool, xpool, tsbpool, spool,
                             X, OUT, cc_in, cc_out,
                             ident_sb, eye3_sb, epseye_sb, eyec, tsb_tiles,
                             use_collective=use_collective)
    return nc


def run_one_pass(nc, tc, cpool, xpool, tsbpool, spool,
                 X, OUT, cc_in, cc_out,
                 ident_sb, eye3_sb, epseye_sb, eyec, tsb_tiles,
                 do_stats=True, do_phase2=True, use_collective=True):
    ident_r = ident_sb[:]
    with (
        tc.tile_pool(name="tp_ps", bufs=1, space="PSUM") as tppool,
        tc.tile_pool(name="acc_ps", bufs=1, space="PSUM") as accpool,
        tc.tile_pool(name="stat_ps", bufs=2, space="PSUM") as stpool,
    ):
        if True:
            # Gram accumulator: [128, 129]; diag 64x64 blocks = per-half
            # Grams, col 128 = per-(half,channel) sums.
            s_psum = accpool.tile([128, C * 2 + 1], F32, tag="sacc")

            xt = [
                xpool.tile([128, HW], F32R, tag=f"xt{p}", name=f"xt{p}")
                for p in range(PAIRS)
            ]
            # bd zero-fill early (bitcast DMA from a zeroed fp32 block) so
            # only the two tiny diag-block DMAs sit on the stats chain.
            bd = cpool.tile([128, 128], F32R, tag="bd")
            zblk = spool.tile([128, 128], F32, tag="zblk")
            nc.vector.memset(zblk[:], 0.0)
            nc.sync.dma_start(out=bd[:], in_=zblk[:].bitcast(F32R))

            # ---------------- phase 1 ----------------
            # groups of up to 4 transpose chunks share one PSUM bank, then one
            # batched copy to bf16 staging, then 1 Gram matmul per chunk.
            groups = []          # (col_offset, [chunk widths])
            for gi in range(6):
                groups.append((gi * 4 * TCH, [TCH] * 4))
            groups.append((24 * TCH, [REM]))
            from concourse.tile_rust import add_dep_helper
            n_mm = 0
            total_mm = PAIRS * (N_FULL + 1)
            gctr = 0
            copy_fns = [nc.vector.tensor_copy, nc.scalar.copy]
            # Chain pair p's load behind pair p-2's completion: keeps ~2 loads
            # in flight (port stays saturated) while forcing near-in-order
            # completion, so the last Gram pair lands ~as early as possible
            # and the AllReduce can trigger sooner.
            pair_dmas = []
            for p in range(PAIRS):
                xs = X[2 * p:2 * p + 2].rearrange("b c s -> (b c) s")
                if p == 0:
                    # split the first tile's load so PE can start sooner
                    nc.sync.dma_start(out=xt[p][:, 0:512], in_=xs[:, 0:512])
                    nc.sync.dma_start(out=xt[p][:, 512:1024],
                                      in_=xs[:, 512:1024])
                    nc.sync.dma_start(out=xt[p][:, 1024:2048],
                                      in_=xs[:, 1024:2048])
                    d = nc.sync.dma_start(out=xt[p][:, 2048:HW],
                                          in_=xs[:, 2048:HW])
                else:
                    d = nc.sync.dma_start(out=xt[p][:], in_=xs)
                    if p >= 2:
                        add_dep_helper(d.ins, pair_dmas[p - 2].ins,
                                       reason="input dma completion order")
                pair_dmas.append(d)
                for go, widths in groups:
                    nchunk = len(widths)
                    tp = tppool.tile([128, 512], F32R, tag=f"tp{gctr % 3}",
                                     name=f"tp_g{gctr % 3}")
                    for j, w in enumerate(widths):
                        nc.tensor.transpose(
                            tp[0:w, j * 128:j * 128 + 128],
                            xt[p][:, go + j * TCH:go + j * TCH + w],
                            ident_r,
                        )
                    tsb = tsb_tiles[gctr % 4]
                    wmin = min(widths)
                    csrc = tp[0:wmin, 0:nchunk * 128].bitcast(F32).rearrange(
                        "p (g c) -> p g c", c=128)
                    cdst = tsb[0:wmin, 0:nchunk * 130].rearrange(
                        "p (g c) -> p g c", c=130)[:, :, 0:128]
                    copy_fns[gctr % 2](cdst, csrc)
                    for j, w in enumerate(widths):
                        b0 = 130 * j
                        nc.tensor.matmul(
                            s_psum[:],
                            lhsT=tsb[0:w, b0:b0 + 128],
                            rhs=tsb[0:w, b0:b0 + 129],
                            start=(n_mm == 0),
                            stop=(n_mm == total_mm - 1),
                        )
                        n_mm += 1
                    gctr += 1

            # ---------------- stats + AllReduce ----------------
            if not do_stats:
                g_dbg = spool.tile([128, C * 2 + 1], F32, tag="gdbg")
                nc.scalar.copy(g_dbg[:], s_psum[:])
                nc.sync.dma_start(out=cc_in.ap()[:, 0:C], in_=g_dbg[0:C, 0:C])
                return
            zout = spool.tile([128, C * 2 + 1], F32, tag="zout")
            nc.scalar.copy(zout[:], s_psum[:])
            # fold bottom half (B^TB block + its sums) onto partitions 0-63
            fold = spool.tile([C, C + 1], F32, tag="fold")
            nc.sync.dma_start(out=fold[:, 0:C], in_=zout[C:2 * C, C:2 * C])
            nc.sync.dma_start(out=fold[:, C:C + 1],
                              in_=zout[C:2 * C, 2 * C:2 * C + 1])
            cc_sb = spool.tile([C, C + 1], F32, tag="cc_sb")
            nc.vector.tensor_add(cc_sb[:, 0:C], zout[0:C, 0:C], fold[:, 0:C])
            nc.vector.tensor_add(cc_sb[:, C:C + 1], zout[0:C, 2 * C:2 * C + 1],
                                 fold[:, C:C + 1])
            d_in = nc.sync.dma_start(out=cc_in.ap(), in_=cc_sb[:])
            from concourse.tile_rust import add_dep_helper
            if use_collective:
                coll = nc.gpsimd.collective_compute(
                    "AllReduce",
                    mybir.AluOpType.add,
                    replica_groups=[list(range(N_CORES))],
                    ins=[cc_in.ap()],
                    outs=[cc_out.ap()],
                )
                add_dep_helper(coll.ins, d_in.ins, reason="collective after input dma")
            else:
                coll = nc.sync.dma_start(out=cc_out.ap(), in_=cc_in.ap())
                add_dep_helper(coll.ins, d_in.ins, reason="collective after input dma")
            g = spool.tile([C, C + 1], F32, tag="g")
            d_out = nc.sync.dma_start(out=g[:], in_=cc_out.ap())
            add_dep_helper(d_out.ins, coll.ins, reason="output dma after collective")

            # mean column, and a copy of it as a row for the outer product
            mcol = spool.tile([C, 1], F32, tag="mcol")
            nc.vector.tensor_scalar_mul(mcol[:], g[:, C:C + 1], 1.0 / M_TOT)
            mrow = spool.tile([1, C], F32, tag="mrow")
            nc.sync.dma_start(out=mrow[:], in_=mcol[:])
            # cov = S - m * outer(mean, mean)
            outer_ps = stpool.tile([C, C], F32, tag="stat")
            nc.tensor.matmul(outer_ps[:], lhsT=mrow[:], rhs=mrow[:],
                             start=True, stop=True)
            cov = spool.tile([C, C], F32, tag="cov")
            nc.vector.scalar_tensor_tensor(
                cov[:], outer_ps[:], -float(M_TOT), g[:, 0:C],
                op0=mybir.AluOpType.mult, op1=mybir.AluOpType.add)
            # per-channel std / rstd
            masked = spool.tile([C, C], F32, tag="masked")
            nc.vector.tensor_tensor(masked[:], cov[:], eyec, mybir.AluOpType.mult)
            var = spool.tile([C, 1], F32, tag="var")
            nc.vector.tensor_reduce(var[:], masked[:], mybir.AxisListType.X,
                                    mybir.AluOpType.add)
            stdv = spool.tile([C, 1], F32, tag="stdv")
            nc.scalar.activation(stdv[:], var[:], mybir.ActivationFunctionType.Sqrt,
                                 scale=1.0 / (M_TOT - 1))
            nc.vector.tensor_scalar_add(stdv[:], stdv[:], EPS_BN)
            rstd = spool.tile([C, 1], F32, tag="rstd")
            nc.vector.reciprocal(rstd[:], stdv[:])
            # sigma = eps*I + diag(rstd) cov diag(rstd) / m
            b1 = spool.tile([C, C], F32, tag="b1")
            nc.vector.tensor_scalar_mul(b1[:], cov[:], rstd[:, 0:1])
            b1t_ps = stpool.tile([C, C], F32, tag="stat")
            nc.tensor.transpose(b1t_ps[:], b1[:], eyec)
            rstd_m = spool.tile([C, 1], F32, tag="rstd_m")
            nc.vector.tensor_scalar_mul(rstd_m[:], rstd[:], 1.0 / M_TOT)
            sigma = spool.tile([C, C], F32, tag="sigma")
            nc.vector.scalar_tensor_tensor(
                sigma[:], b1t_ps[:], rstd_m[:, 0:1], epseye_sb[:],
                op0=mybir.AluOpType.mult, op1=mybir.AluOpType.add)

            # Newton-Schulz: Y0=sigma, Z0=I;  T=3I-ZY; Y<-0.5*Y@T; Z<-0.5*T@Z
            t1 = spool.tile([C, C], F32, tag="ns_t0")
            nc.vector.tensor_sub(t1[:], eye3_sb[:], sigma[:])
            y = spool.tile([C, C], F32, tag="ns_y0")
            y_ps = stpool.tile([C, C], F32, tag="stat")
            nc.tensor.matmul(y_ps[:], lhsT=sigma[:], rhs=t1[:], start=True, stop=True)
            nc.scalar.activation(y[:], y_ps[:], mybir.ActivationFunctionType.Copy,
                                 scale=0.5)
            z = spool.tile([C, C], F32, tag="ns_z0")
            nc.scalar.mul(z[:], t1[:], 0.5)
            for k in range(1, NS_ITERS):
                p_ps = stpool.tile([C, C], F32, tag="stat")
                nc.tensor.matmul(p_ps[:], lhsT=z[:], rhs=y[:], start=True, stop=True)
                tk = spool.tile([C, C], F32, tag=f"ns_t{k}")
                nc.vector.tensor_sub(tk[:], eye3_sb[:], p_ps[:])
                zn = spool.tile([C, C], F32, tag=f"ns_z{k}")
                z_ps = stpool.tile([C, C], F32, tag="stat")
                nc.tensor.matmul(z_ps[:], lhsT=tk[:], rhs=z[:], start=True, stop=True)
                nc.scalar.activation(zn[:], z_ps[:],
                                     mybir.ActivationFunctionType.Copy, scale=0.5)
                if k < NS_ITERS - 1:
                    yn = spool.tile([C, C], F32, tag=f"ns_y{k}")
                    yn_ps = stpool.tile([C, C], F32, tag="stat")
                    nc.tensor.matmul(yn_ps[:], lhsT=y[:], rhs=tk[:],
                                     start=True, stop=True)
                    nc.scalar.activation(yn[:], yn_ps[:],
                                         mybir.ActivationFunctionType.Copy,
                                         scale=0.5)
                    y = yn
                z = zn
            wm = z

            # A^T = diag(rstd) @ wm ; block-diag lhsT; bias
            at = spool.tile([C, C], F32, tag="at")
            nc.vector.tensor_scalar_mul(at[:], wm[:], rstd[:, 0:1])
            at_r = at[:].bitcast(F32R)
            nc.sync.dma_start(out=bd[0:C, 0:C], in_=at_r)
            nc.sync.dma_start(out=bd[C:2 * C, C:2 * C], in_=at_r)

            v = spool.tile([C, 1], F32, tag="v")
            nc.vector.tensor_scalar(v[:], mcol[:], rstd[:, 0:1], -1.0,
                                    op0=mybir.AluOpType.mult,
                                    op1=mybir.AluOpType.mult)
            bias_ps = stpool.tile([C, 1], F32, tag="stat")
            nc.tensor.matmul(bias_ps[:], lhsT=wm[:], rhs=v[:], start=True, stop=True)
            bias_sb = spool.tile([C, 1], F32, tag="bias")
            nc.scalar.copy(bias_sb[:], bias_ps[:])
            bias2 = spool.tile([128, 1], F32, tag="bias2")
            nc.sync.dma_start(out=bias2[0:C, :], in_=bias_sb[:])
            nc.sync.dma_start(out=bias2[C:2 * C, :], in_=bias_sb[:])

    # ---------------- phase 2 ----------------
    if not do_phase2:
        return
    bd_r = bd[:]
    with (
        tc.tile_pool(name="outs", bufs=3) as opool,
        tc.tile_pool(name="p2_ps", bufs=4, space="PSUM") as p2pool,
    ):
            for p in range(PAIRS):
                osb = opool.tile([128, HW], F32, tag="osb")
                for ci, (o, w) in enumerate(P2_CHUNKS):
                    po = p2pool.tile([128, OCH], F32, tag="p2")
                    nc.tensor.matmul(po[:, 0:w], lhsT=bd_r,
                                     rhs=xt[p][:, o:o + w],
                                     start=True, stop=True)
                    if ci % 2 == 0:
                        nc.scalar.activation(osb[:, o:o + w], po[:, 0:w],
                                             mybir.ActivationFunctionType.Identity,
                                             bias=bias2[:, 0:1], scale=1.0)
                    else:
                        nc.vector.tensor_scalar_add(osb[:, o:o + w], po[:, 0:w],
                                                    bias2[:, 0:1])
                odst = OUT[2 * p:2 * p + 2].rearrange("b c s -> (b c) s")
                nc.sync.dma_start(out=odst[:, 0:1536], in_=osb[:, 0:1536])
                nc.sync.dma_start(out=odst[:, 1536:HW], in_=osb[:, 1536:HW])


_NC_CACHE = None


def _get_nc():
    global _NC_CACHE
    if _NC_CACHE is None:
        _NC_CACHE = build_bass()
    return _NC_CACHE


_RUNNER = None


def _get_runner():
    """Build (once) a jitted shard_map runner over the 8 cores with the
    constant inputs and output scratch kept device-resident."""
    global _RUNNER
    if _RUNNER is not None:
        return _RUNNER
    import jax
    from jax.sharding import Mesh, PartitionSpec
    from jax.experimental.shard_map import shard_map
    from concourse import bass2jax

    nc = _get_nc()
    bass2jax.install_neuronx_cc_hook()
    partition_name = nc.partition_id_tensor.name if nc.partition_id_tensor else None
    in_names, out_names, out_avals, zero_outs = [], [], [], []
    for alloc in nc.m.functions[0].allocations:
        if not isinstance(alloc, mybir.MemoryLocationSet):
            continue
        name = alloc.memorylocations[0].name
        if alloc.kind == "ExternalInput":
            if name != partition_name:
                in_names.append(name)
        elif alloc.kind == "ExternalOutput":
            shape = tuple(alloc.tensor_shape)
            dtype = mybir.dt.np(alloc.dtype)
            out_names.append(name)
            out_avals.append(jax.core.ShapedArray(shape, dtype))
            zero_outs.append(np.zeros(shape, dtype))
    n_params = len(in_names)
    in_names_all = in_names + out_names
    if partition_name is not None:
        in_names_all.append(partition_name)

    def _body(*args):
        operands = list(args)
        if partition_name is not None:
            operands.append(bass2jax.partition_id_tensor())
        outs = bass2jax._bass_exec_p.bind(
            *operands,
            out_avals=tuple(out_avals),
            in_names=tuple(in_names_all),
            out_names=tuple(out_names),
            lowering_input_output_aliases=(),
            sim_require_finite=True,
            sim_require_nnan=True,
            nc=nc,
        )
        return tuple(outs)

    devices = jax.devices()[:N_CORES]
    mesh = Mesh(np.asarray(devices), ("core",))
    n_outs = len(out_avals)
    in_specs = (PartitionSpec("core"),) * (n_params + n_outs)
    out_specs = (PartitionSpec("core"),) * n_outs
    sharded = jax.jit(
        shard_map(_body, mesh=mesh, in_specs=in_specs, out_specs=out_specs,
                  check_rep=False),
        keep_unused=True,
    )
    consts = {
        "IDENT": np.eye(128, dtype=np.float32),
        "EYE3": 3.0 * np.eye(C, dtype=np.float32),
        "EPSEYE": EPS * np.eye(C, dtype=np.float32),
    }
    dev_consts = {}
    for name in in_names:
        if name in consts:
            dev_consts[name] = jax.device_put(
                np.concatenate([consts[name]] * N_CORES, axis=0))
    dev_zeros = [
        jax.device_put(np.zeros((N_CORES * z.shape[0], *z.shape[1:]), z.dtype))
        for z in zero_outs
    ]
    _RUNNER = (sharded, in_names, out_names, out_avals, dev_consts, dev_zeros)
    return _RUNNER


def kernel(X: np.ndarray) -> np.ndarray:
    X = np.asarray(X)
    assert X.shape == (B, C, H, W) and X.dtype == np.float32
    sharded, in_names, out_names, out_avals, dev_consts, dev_zeros = _get_runner()
    xr = np.ascontiguousarray(X.reshape(B, C, HW))
    args = []
    for name in in_names:
        if name == "X":
            args.append(xr)
        else:
            args.append(dev_consts[name])
    args.extend(dev_zeros)
    out_arrs = sharded(*args)
    oi = out_names.index("OUT")
    out = np.asarray(out_arrs[oi])
    return np.ascontiguousarray(out.reshape(B, C, H, W))
